# revision 53
# baseline (speedup 1.0000x reference)
"""Trainium2 Bass kernel for causal self-attention with GQA + RoPE.

Model: B=2, T=2048, C=2048, H=16 query heads, H_KV=4 kv heads, D=128.

Sharding (8 NeuronCores, pure SPMD, no collectives):
  core i -> batch b = i // 4, kv-group g = i % 4
            (query heads 4g..4g+3, kv head g, all T positions of batch b).
  Every core runs an identical program; only input data differs.
  o_proj is computed against the row-slice wo[512g:512(g+1), :], giving a
  partial [T, C] output per core; the sum over the 4 cores of each batch
  (the tensor-parallel all-reduce) is done on the host in numpy.

v3 (on top of the bf16 v2 baseline):
  - rowsum pair-reduce: adjacent 128-wide k-subtiles of exp(S) are summed
    pairwise on DVE (bf16 tensor_tensor, 2x mode); the ones-matmul rowsum
    then streams half the columns (PE 29us -> 16us). Cross-pair
    accumulation stays in PSUM fp32 (the single bf16 pair-add rounds
    independently per element -> rowsum error ~2^-9/sqrt(1024), trivial).
    The final (diagonal) group of each head keeps direct rowsum matmuls
    so no cross-head deferral is needed.
  - o_proj PSUM evacuation alternates vector/scalar (was all-vector, which
    serialized the o_unit pipeline through one engine).
  - tail restructure: the attention-only PSUM pools close before the last
    chunk's o_proj, freeing 6 banks; the tail runs from a 4-deep PSUM pool
    with stores on sync/scalar. gpsimd (slow ~7.6us software-DGE drain)
    issues no DMA after mid-attention.

v4 (v4.1 after the big-HWDGE regression):
  - wk|wv concatenated host-side into one wkv tensor (one descriptor per
    cc on gpsimd's software-DGE queue, which issues at ~0.75us/descriptor
    but keeps pace with per-cc K/V consumption).
  - phase-0 x/wq loads as per-cc singles, cc-interleaved across the two
    hardware-DGE queues (sync/scalar) in consumption order. (Big grouped
    transfers on the HW-DGE rings measured ~35GB/s - they do NOT stripe
    across SDMA engines - while a pipelined stream of 128KB descriptors
    sustains ~90GB/s/queue. gpsimd's SWDGE handles grouped transfers
    fine, so bulk non-critical bytes - rope tables, wo - go there.)
  - x for q-chunks 1..3 prefetched one chunk ahead (8 slices into a
    double-buffered tile, alternating sync/scalar).
  - rope restructure: the six PSUM evacuations of a chunk's projections
    are emitted back-to-back at chunk end, split scalar/vector; the rope
    rotate-matmuls + V transposes of chunk qc are interleaved into chunk
    qc+1's projection stream (the PE never waits on the evac/TT chain);
    the rotate/transpose PSUM tag is double-buffered. This removes the
    ~2-3us PE stall at every chunk boundary that also re-throttled the
    PE clock (HAM) mid-kernel.
  - the LAST chunk's rope/V-transpose is deferred into attention chunk 0
    as PE filler (rotate-PSUM from the then-idle o_proj pool), removing
    the phase-transition stall cluster.
"""

import math
import os

import numpy as np

os.environ.setdefault("MYCRO_LOCAL_CACHE", "1")

P = 128
D = 128
H = 16
H_KV = 4
GQ = H // H_KV  # 4 query heads per kv head (= per core)
B = 2
T_FULL = 2048
C_DIM = 2048
NCORES = 8
ROPE_BASE = 10000.0


def _rope_tables(T):
    inv_freq = 1.0 / (ROPE_BASE ** (np.arange(0, D, 2, dtype=np.float32) / D))
    t = np.arange(T, dtype=np.float32)
    freqs = np.outer(t, inv_freq)  # [T, D/2]
    emb = np.concatenate((freqs, freqs), axis=-1)  # [T, D]
    return (
        np.ascontiguousarray(np.cos(emb).T.astype(np.float32)),  # [D, T]
        np.ascontiguousarray(np.sin(emb).T.astype(np.float32)),
    )


def _rot_lhsT():
    # rotate_half(q) = R @ q with R[d, d+64] = -1 (d < 64), R[d, d-64] = +1.
    # matmul computes lhsT.T @ rhs, so pass lhsT = R^T.
    R = np.zeros((D, D), dtype=np.float32)
    half = D // 2
    R[np.arange(half), np.arange(half) + half] = -1.0
    R[np.arange(half) + half, np.arange(half)] = 1.0
    return np.ascontiguousarray(R.T)


def _tri128():
    # tri[k, j] = 1 if j >= k else 0: the in-subtile causal triangle after
    # diagonal narrowing (column j of a narrowed diag slice is q = 128m + j,
    # row k is k_local; valid iff j >= k).
    k = np.arange(P)
    return (k[None, :] >= k[:, None]).astype(np.float32)


def build_nc(T=T_FULL):
    """Build the per-core Bass/Tile program (identical across cores)."""
    from contextlib import ExitStack

    import concourse.mybir as mybir
    import concourse.tile as tile
    from concourse import bacc
    from concourse.masks import make_identity

    f32 = mybir.dt.float32
    bf16 = mybir.dt.bfloat16
    Exp = mybir.ActivationFunctionType.Exp
    MULT = mybir.AluOpType.mult
    ADD = mybir.AluOpType.add
    SCALE = 1.0 / math.sqrt(D)

    NCC = C_DIM // P  # 16 contraction chunks
    NQC = T // 512  # projection / attention q-chunks (512-wide)
    NCT = C_DIM // 512  # o_proj column tiles
    NKB = T // P  # 128-wide k subtiles

    nc = bacc.Bacc(
        "TRN2",
        target_bir_lowering=False,
        debug=False,
        num_devices=NCORES,
    )

    xt = nc.dram_tensor("xt", [C_DIM, T], bf16, kind="ExternalInput").ap()
    wq = nc.dram_tensor("wq", [C_DIM, GQ * D], bf16, kind="ExternalInput").ap()
    wkv = nc.dram_tensor("wkv", [C_DIM, 2 * D], bf16, kind="ExternalInput").ap()
    wo = nc.dram_tensor("wo", [GQ * D, C_DIM], bf16, kind="ExternalInput").ap()
    cosT = nc.dram_tensor("cosT", [D, T], bf16, kind="ExternalInput").ap()
    sinT = nc.dram_tensor("sinT", [D, T], bf16, kind="ExternalInput").ap()
    trim = nc.dram_tensor("trim", [P, P], bf16, kind="ExternalInput").ap()
    onesm = nc.dram_tensor("onesm", [P, P], bf16, kind="ExternalInput").ap()
    rotm = nc.dram_tensor("rotm", [P, P], bf16, kind="ExternalInput").ap()
    out = nc.dram_tensor("out", [T, C_DIM], bf16, kind="ExternalOutput").ap()

    with tile.TileContext(nc) as tc, ExitStack() as ctx:
        const = ctx.enter_context(tc.tile_pool(name="const", bufs=1))
        acts = ctx.enter_context(tc.tile_pool(name="acts", bufs=1))

        wq_r = wq.rearrange("(cc p) n -> p cc n", p=P)
        wkv_r = wkv.rearrange("(cc p) n -> p cc n", p=P)
        xt_r = xt.rearrange("(cc p) t -> p cc t", p=P)
        wo_r = wo.rearrange("(h p) (ct n) -> p h ct n", p=P, n=512)

        ones_sb = const.tile([P, P], bf16)
        rot_sb = const.tile([P, P], bf16)
        ident = const.tile([P, P], bf16)
        tri_sb = const.tile([P, P], bf16)

        # long-lived activations (all bf16: ~60KB/partition total)
        qt_sb = [acts.tile([P, T], bf16, name=f"qt{h}") for h in range(GQ)]
        kt_sb = acts.tile([P, T], bf16, name="kt")
        v_sb = acts.tile([P, NKB, D], bf16, name="vnat")
        y_sb = [acts.tile([P, T], bf16, name=f"yt{h}") for h in range(GQ)]
        wo_sb = acts.tile([P, GQ, NCT, 512], bf16, name="wo_sb")
        cos_sb = acts.tile([P, T], bf16, name="cos_sb")
        sin_sb = acts.tile([P, T], bf16, name="sin_sb")
        # last-chunk projection evacuations, consumed in phase 2
        rawL = [acts.tile([P, 512], bf16, name=f"rawL{i}") for i in range(6)]

        # ---------------- phase 1: projections + rope ----------------
        with (
            tc.tile_pool(name="pwts", bufs=1) as wpool,
            tc.tile_pool(name="xts", bufs=4) as xt_pool,
            tc.tile_pool(name="rope_t", bufs=1) as rope_pool,
            tc.tile_pool(name="proj_ps", bufs=1, space="PSUM") as proj_ps,
            tc.tile_pool(name="aux_ps", bufs=1, space="PSUM") as aux_ps,
            tc.tile_pool(name="ptmp", bufs=2) as ptmp,
        ):
            wq_sb = wpool.tile([P, NCC, GQ * D], bf16)
            wkv_sb = wpool.tile([P, NCC, 2 * D], bf16)
            lead_xs = xt_pool.tile([P, NCC, 512], bf16, tag="xlead",
                               name="lead_xs", bufs=1)

            # identity first: two cheap gpsimd ops, then gpsimd is free to
            # issue DMA descriptors.
            make_identity(nc, ident)

            # Phase-0 loads as per-cc singles (the HW-DGE rings pipeline
            # 128KB descriptors at ~90GB/s; larger transfers are NOT
            # faster - they don't stripe across SDMA engines). A dma_start
            # blocks its ISSUING ENGINE while the ring is full (depth ~5),
            # and everything behind it in that engine's FIFO stalls too -
            # so scalar (which must run the PSUM evacuation copies at
            # chunk end) gets few posts, sync (no compute duties) carries
            # most of x, and gpsimd's SWDGE carries wkv + the wq tail +
            # all bulk non-urgent bytes.
            # cc0's x/wq land as 64KB halves so the very first matmuls can
            # start ~1us sooner after the fixed ~7.2us framework preamble
            # (cc0's projection runs as two half-width rhs passes).
            nc.sync.dma_start(lead_xs[:, 0, 0:256], xt_r[:, 0, 0:256])
            nc.sync.dma_start(lead_xs[:, 0, 256:512], xt_r[:, 0, 256:512])
            for cc in range(1, 13):
                nc.sync.dma_start(lead_xs[:, cc, :], xt_r[:, cc, 0:512])
            nc.scalar.dma_start(wq_sb[:, 0, 0:256], wq_r[:, 0, 0:256])
            nc.scalar.dma_start(wq_sb[:, 0, 256:512], wq_r[:, 0, 256:512])
            for cc in range(1, 10):
                nc.scalar.dma_start(wq_sb[:, cc, :], wq_r[:, cc, :])
            for cc in range(13, NCC):
                nc.scalar.dma_start(lead_xs[:, cc, :], xt_r[:, cc, 0:512])
            for cc in range(6):
                nc.gpsimd.dma_start(wkv_sb[:, cc, :], wkv_r[:, cc, :])
            for j in range(6):
                nc.gpsimd.dma_start(wq_sb[:, 10 + j, :], wq_r[:, 10 + j, :])
                nc.gpsimd.dma_start(wkv_sb[:, 6 + j, :], wkv_r[:, 6 + j, :])
            for cc in range(12, NCC):
                nc.gpsimd.dma_start(wkv_sb[:, cc, :], wkv_r[:, cc, :])
            # rope tables / consts on gpsimd (rotm + chunk-0 tables needed
            # from ~33us). The BULK (rope-table tails, wo: 2.75MB, needed
            # only from ~60us) is emitted later behind a data gate - the
            # SDMA engines round-robin between queues at packet granularity
            # and these transfers would otherwise steal HBM bandwidth from
            # the critical phase-0/prefetch tail.
            nc.gpsimd.dma_start(rot_sb[:], rotm)
            nc.gpsimd.dma_start(cos_sb[:, 0:512], cosT[:, 0:512])
            nc.gpsimd.dma_start(sin_sb[:, 0:512], sinT[:, 0:512])
            nc.gpsimd.dma_start(tri_sb[:], trim)
            nc.gpsimd.dma_start(ones_sb[:], onesm)
            # warm the ACT exp table set during the initial DMA wait
            warm = ptmp.tile([P, 1], f32, name="warm", tag="warm")
            nc.scalar.activation(warm[:], warm[:], Exp)

            def rot_tt(raw, dst, cosq, sinq):
                # dst = raw*cos + (R raw)*sin
                rp = aux_ps.tile([P, 512], f32, name="rotp", tag="rotp",
                                 bufs=2)
                nc.tensor.matmul(rp[:], rot_sb[:], raw[:], start=True,
                                 stop=True)
                nc.vector.tensor_tensor(dst, raw[:], cosq, MULT)
                t2 = ptmp.tile([P, 512], bf16, name="rt2", tag="rt2")
                nc.vector.tensor_tensor(t2[:], rp[:], sinq, MULT)
                nc.vector.tensor_tensor(dst, dst, t2[:], ADD)

            # pending rope work of the previous chunk, interleaved into the
            # current chunk's projection stream so the PE never waits on
            # the PSUM-evacuation/TT chain.
            pend_rope = None  # (qc_prev, raws[4], rawk, vraw)
            xh_next = None

            for qc in range(NQC):
                q0 = qc * 512
                if qc == 0:
                    xt_all = lead_xs
                else:
                    xt_all = xh_next
                xh = None
                if qc + 1 < NQC:
                    xh = xt_pool.tile([P, NCC, 512], bf16, tag="xh",
                                      name="xh", bufs=2)

                qp = [
                    proj_ps.tile([P, 512], f32, name=f"qp{h}", tag=f"qp{h}")
                    for h in range(GQ)
                ]
                kp = proj_ps.tile([P, 512], f32, name="kp", tag="kp")
                vp = proj_ps.tile([P, 512], f32, name="vp", tag="vp")
                for cc in range(NCC):
                    xtile = xt_all[:, cc, :]
                    first, last = cc == 0, cc == NCC - 1
                    if qc == 0 and cc == 0:
                        # chunk 0 / cc 0 runs as two half-width rhs passes,
                        # ordered so each matmul only needs the 64KB half
                        # transfers that have already landed (h0/h1
                        # weights + k/v in the first halves). Only the
                        # FIRST half carries start=True: start clears the
                        # whole bank's has_written bits, and the second
                        # half then overwrites its (unwritten) region
                        # under flags=0 semantics.
                        for c0, c1 in ((0, 256), (256, 512)):
                            st = c0 == 0
                            xh_ = lead_xs[:, 0, c0:c1]
                            for h in (0, 1):
                                nc.tensor.matmul(
                                    qp[h][:, c0:c1],
                                    wq_sb[:, 0, h * D : (h + 1) * D],
                                    xh_, start=st, stop=False,
                                )
                            nc.tensor.matmul(
                                kp[:, c0:c1], wkv_sb[:, 0, 0:D], xh_,
                                start=st, stop=False,
                            )
                            nc.tensor.matmul(
                                vp[:, c0:c1], wkv_sb[:, 0, D : 2 * D], xh_,
                                start=st, stop=False,
                            )
                            for h in (2, 3):
                                nc.tensor.matmul(
                                    qp[h][:, c0:c1],
                                    wq_sb[:, 0, h * D : (h + 1) * D],
                                    xh_, start=st, stop=False,
                                )
                        continue
                    for h in range(GQ):
                        nc.tensor.matmul(
                            qp[h][:],
                            wq_sb[:, cc, h * D : (h + 1) * D],
                            xtile,
                            start=first,
                            stop=last,
                        )
                    nc.tensor.matmul(
                        kp[:], wkv_sb[:, cc, 0:D], xtile, start=first,
                        stop=last
                    )
                    nc.tensor.matmul(
                        vp[:], wkv_sb[:, cc, D : 2 * D], xtile, start=first,
                        stop=last
                    )
                    # previous chunk's rope/V-transpose work as filler
                    if pend_rope is not None and cc in (0, 1, 2, 4, 5):
                        pq, raws, rawk, vraw = pend_rope
                        pq0 = pq * 512
                        pcos = cos_sb[:, pq0 : pq0 + 512]
                        psin = sin_sb[:, pq0 : pq0 + 512]
                        if cc == 0:
                            rot_tt(raws[0], qt_sb[0][:, pq0 : pq0 + 512],
                                   pcos, psin)
                            rot_tt(raws[1], qt_sb[1][:, pq0 : pq0 + 512],
                                   pcos, psin)
                        elif cc == 1:
                            rot_tt(raws[2], qt_sb[2][:, pq0 : pq0 + 512],
                                   pcos, psin)
                            rot_tt(raws[3], qt_sb[3][:, pq0 : pq0 + 512],
                                   pcos, psin)
                        elif cc == 2:
                            rot_tt(rawk, kt_sb[:, pq0 : pq0 + 512],
                                   pcos, psin)
                        elif cc in (4, 5):
                            for ks in ((0, 1) if cc == 4 else (2, 3)):
                                tp = aux_ps.tile([P, P], bf16, name="vtrp",
                                                 tag="rotp", bufs=2)
                                nc.tensor.transpose(
                                    tp[:], vraw[:, ks * P : (ks + 1) * P],
                                    ident[:],
                                )
                                nc.vector.tensor_copy(
                                    v_sb[:, pq * 4 + ks, :], tp[:]
                                )

                # end of chunk: evacuate all six projection accumulators
                # into the long-lived rawL tiles - all on VECTOR, whose
                # FIFO has no DMA posts (a ring-full dma_start parked on
                # scalar would delay any evac behind it by many us). The
                # rope matmuls run inside the next chunk's projections (or,
                # for the last chunk, as attention filler in phase 2).
                # Exception: the LAST chunk splits scalar/vector - scalar's
                # posts are long done by then, and phase 2's PSUM pools
                # can't allocate until this whole wave completes (the pool
                # boundary waits on the phase-1 PSUM pools' release).
                if qc == NQC - 1:
                    for h in range(GQ):
                        if h % 2 == 0:
                            nc.scalar.copy(rawL[h][:], qp[h][:])
                        else:
                            nc.vector.tensor_copy(rawL[h][:], qp[h][:])
                    nc.scalar.copy(rawL[4][:], kp[:])
                    nc.vector.tensor_copy(rawL[5][:], vp[:])
                else:
                    for h in range(GQ):
                        nc.vector.tensor_copy(rawL[h][:], qp[h][:])
                    nc.vector.tensor_copy(rawL[4][:], kp[:])
                    nc.vector.tensor_copy(rawL[5][:], vp[:])
                pend_rope = (qc, rawL[0:4], rawL[4], rawL[5])
                # next chunk's x prefetch posts go out AFTER the evacuation
                # copies (engine FIFOs execute in emission order, and a
                # ring-full dma_start would block the evacs behind it).
                if xh is not None:
                    nq0 = (qc + 1) * 512
                    for xg in range(8):
                        q_ = (nc.sync, nc.scalar)[xg % 2]
                        q_.dma_start(
                            xh[:, 2 * xg : 2 * xg + 2, :],
                            xt_r[:, 2 * xg : 2 * xg + 2, nq0 : nq0 + 512],
                        )
                if qc == 0:
                    # gate: this copy waits for chunk-1's last x slice, so
                    # the bulk posts behind it in gpsimd's FIFO can't start
                    # transferring until the critical window has passed.
                    gate = ptmp.tile([P, 4], bf16, name="gate", tag="gate")
                    nc.gpsimd.tensor_copy(gate[:], xh[:, 15, 0:4])
                    nc.gpsimd.dma_start(cos_sb[:, 512:T], cosT[:, 512:T])
                    nc.gpsimd.dma_start(sin_sb[:, 512:T], sinT[:, 512:T])
                    # wo preload (needed ~115us) as two 1MB SWDGE transfers
                    nc.gpsimd.dma_start(wo_sb[:, 0:2, :, :],
                                        wo_r[:, 0:2, :, :])
                    nc.gpsimd.dma_start(wo_sb[:, 2:4, :, :],
                                        wo_r[:, 2:4, :, :])
                xh_next = xh

        # -------- phase 2: causal attention + interleaved o_proj --------
        with (
            tc.tile_pool(name="pt_pool", bufs=3) as pt_pool,
            tc.tile_pool(name="o_ps", bufs=2, space="PSUM") as o_ps,
            tc.tile_pool(name="nrm", bufs=2) as nrm_pool,
            tc.tile_pool(name="ost", bufs=4) as ost_pool,
            tc.tile_pool(name="pairs", bufs=3) as pair_pool,
        ):
            o_count = [0]
            o_queues = (nc.sync, nc.scalar, nc.gpsimd)
            evac_engines = (nc.vector, nc.scalar)

            def o_finish(op, ct, qb, store_queues, ev=None):
                ot = ost_pool.tile([P, 512], bf16, name="ot", tag="ot",
                                   bufs=8)
                if ev is None:
                    ev = evac_engines[o_count[0] % 2]
                if ev is nc.scalar:
                    nc.scalar.copy(ot[:], op[:])
                else:
                    nc.vector.tensor_copy(ot[:], op[:])
                oq = store_queues[o_count[0] % len(store_queues)]
                o_count[0] += 1
                oq.dma_start(
                    out[qb * P : (qb + 1) * P, ct * 512 : (ct + 1) * 512],
                    ot[:],
                )

            def o_unit(aq, ct, qb, ps_pool, store_queues, ev=None):
                # one o_proj output tile [128 q rows, 512 cols] for chunk aq
                op = ps_pool.tile([P, 512], f32, name="op", tag="op")
                for h in range(GQ):
                    nc.tensor.matmul(
                        op[:],
                        y_sb[h][:, qb * P : (qb + 1) * P],
                        wo_sb[:, h, ct, :],
                        start=(h == 0),
                        stop=(h == GQ - 1),
                    )
                o_finish(op, ct, qb, store_queues, ev)

            def make_units(aq):
                return [(aq, ct, qb) for ct in range(NCT)
                        for qb in range(4 * aq, 4 * aq + 4)]

            # deferred rope/V-transpose of the last projection chunk,
            # executed as PE filler inside attention chunk 0 (the o_proj
            # pool is idle there, so its PSUM banks host the rotate
            # matmuls / transposes).
            pq, praws, prawk, pvraw = pend_rope
            pq0 = pq * 512
            pcos = cos_sb[:, pq0 : pq0 + 512]
            psin = sin_sb[:, pq0 : pq0 + 512]

            def d_rot(raw, dst):
                def fn():
                    rp = o_ps.tile([P, 512], f32, name="rpd", tag="op")
                    nc.tensor.matmul(rp[:], rot_sb[:], raw[:], start=True,
                                     stop=True)
                    nc.vector.tensor_tensor(dst, raw[:], pcos, MULT)
                    t2 = nrm_pool.tile([P, 512], bf16, name="rt2d",
                                       tag="rt2d")
                    nc.vector.tensor_tensor(t2[:], rp[:], psin, MULT)
                    nc.vector.tensor_tensor(dst, dst, t2[:], ADD)
                return fn

            def d_vt(ks0, ks1):
                def fn():
                    for ks in (ks0, ks1):
                        tp = o_ps.tile([P, P], bf16, name="vtpd", tag="op")
                        nc.tensor.transpose(
                            tp[:], pvraw[:, ks * P : (ks + 1) * P], ident[:]
                        )
                        nc.vector.tensor_copy(v_sb[:, pq * 4 + ks, :], tp[:])
                return fn

            rope_fill = [d_rot(praws[h], qt_sb[h][:, pq0 : pq0 + 512])
                         for h in range(GQ)]
            rope_fill.append(d_rot(prawk, kt_sb[:, pq0 : pq0 + 512]))
            rope_fill.append(d_vt(0, 1))
            rope_fill.append(d_vt(2, 3))

            with (
                tc.tile_pool(name="s_ps", bufs=2, space="PSUM") as s_ps,
                tc.tile_pool(name="y_ps", bufs=1, space="PSUM") as y_ps,
                tc.tile_pool(name="rs_ps", bufs=1, space="PSUM") as rs_ps,
            ):
                for aq in range(NQC):
                    q0 = aq * 512
                    nks = 4 * (aq + 1)  # 128-wide k subtiles (incl 4 diagonal)
                    ng = nks // 2  # groups of 2 subtiles
                    units = make_units(aq - 1) if aq > 0 else []
                    slots = GQ * ng
                    credit = 0.0
                    ucount = len(units)

                    # narrowed (offset, width) per k-subtile: diagonal subtile
                    # m only covers q >= 128m within the 512-wide chunk.
                    def ow(ks):
                        m = ks - (nks - 4)
                        if m > 0:
                            return 128 * m, 512 - 128 * m
                        return 0, 512

                    for h in range(GQ):
                        qrow = qt_sb[h]
                        yp = y_ps.tile([P, 512], f32, name="yp", tag="yp")
                        rp_ = rs_ps.tile([P, 512], f32, name="rsp", tag="rsp")
                        sps = [None] * ng
                        # pair tiles awaiting their rowsum matmul:
                        # list of (tile, offA) in group order
                        pend_pairs = [None] * ng

                        def s_issue(g):
                            # the two subtiles are packed back to back in the
                            # sp tile ([0:w0], [w0:w0+w1]); w0 is always 256
                            # or 512 so neither matmul output crosses a PSUM
                            # bank.
                            sp = s_ps.tile([P, 1024], f32, name="sp", tag="sp")
                            off1 = 0
                            for ks in (2 * g, 2 * g + 1):
                                off, w = ow(ks)
                                nc.tensor.matmul(
                                    sp[:, off1 : off1 + w],
                                    kt_sb[:, ks * P : (ks + 1) * P],
                                    qrow[:, q0 + off : q0 + 512],
                                    start=True,
                                    stop=True,
                                )
                                off1 += w
                            sps[g] = sp

                        s_issue(0)
                        if ng > 1:
                            s_issue(1)
                        # full (non-diagonal) pairs are quad-reduced: pairs
                        # (2j, 2j+1) of groups 0..ng-3 sum on DVE into one
                        # quad tile whose rowsum matmul streams 512 cols
                        # for FOUR k-subtiles. The diagonal pair (group
                        # ng-2) and the final direct group keep the v3
                        # scheme. rp_'s first writer is quad 0 (start=True)
                        # at group 3, or the diagonal pair for aq==0.
                        pend_quads = {}  # emit_group -> quad tile
                        for g in range(ng):
                            if g + 2 < ng:
                                s_issue(g + 2)
                            # rowsum matmuls whose DVE reductions have had
                            # >= 2 groups to finish: quads scheduled for
                            # this group, then (at the last group) the
                            # diagonal pair.
                            if g in pend_quads:
                                qd, j = pend_quads.pop(g)
                                nc.tensor.matmul(
                                    rp_[:, 0:512],
                                    ones_sb[:],
                                    qd[:, 0:512],
                                    start=(j == 0),
                                    stop=False,
                                )
                            if g == ng - 1 and pend_pairs[g - 1] is not None:
                                pr, poff = pend_pairs[g - 1]
                                nc.tensor.matmul(
                                    rp_[:, poff:512],
                                    ones_sb[:],
                                    pr[:, poff:512],
                                    start=(ng == 2),
                                    stop=False,
                                )
                                pend_pairs[g - 1] = None
                            # deferred last-chunk rope as PE filler (aq 0)
                            if rope_fill:
                                rope_fill.pop(0)()
                            # o_proj filler for the previous q-chunk. The
                            # last few units before the attention->o_proj
                            # tail handoff evacuate on scalar, keeping the
                            # DVE queue clear for the final head's
                            # rowsum-reciprocal/normalize chain.
                            credit += ucount / slots
                            while credit >= 1.0 and units:
                                ev = (nc.scalar if (aq == NQC - 1
                                                    and len(units) <= 4)
                                      else None)
                                o_unit(*units.pop(0), o_ps, o_queues, ev)
                                credit -= 1.0
                            sp = sps[g]
                            pt = pt_pool.tile([P, 1024], bf16, name="ptile",
                                              tag="pt")
                            subs = (2 * g, 2 * g + 1)
                            (offA, wA), (offB, wB) = ow(subs[0]), ow(subs[1])
                            wsum = wA + wB
                            nc.scalar.activation(
                                pt[:, 0:wsum], sp[:, 0:wsum], Exp, scale=SCALE
                            )
                            off1 = 0
                            for ks in subs:
                                w = ow(ks)[1]
                                if ks - (nks - 4) >= 0:
                                    # causal triangle on the first 128 cols
                                    # of the narrowed slice
                                    sl = pt[:, off1 : off1 + P]
                                    nc.vector.tensor_tensor(sl, sl, tri_sb[:],
                                                            MULT)
                                off1 += w
                            last_group = g == ng - 1
                            if not last_group:
                                # pair-reduce the two subtiles on DVE (bf16,
                                # one rounding per element).
                                pair = pair_pool.tile([P, 512], bf16,
                                                      name="pair", tag="pair")
                                if offB > offA:
                                    # diagonal pair: [offA:offB] has only A
                                    nc.vector.tensor_copy(
                                        pair[:, offA:offB],
                                        pt[:, 0 : offB - offA],
                                    )
                                    nc.vector.tensor_tensor(
                                        pair[:, offB:512],
                                        pt[:, offB - offA : wA],
                                        pt[:, wA : wA + wB],
                                        ADD,
                                    )
                                else:
                                    nc.vector.tensor_tensor(
                                        pair[:, 0:512],
                                        pt[:, 0:512],
                                        pt[:, 512:1024],
                                        ADD,
                                    )
                                pend_pairs[g] = (pair, offA)
                                if g % 2 == 1 and g < ng - 2:
                                    # quad: sum the two full pairs on DVE
                                    quad = pair_pool.tile(
                                        [P, 512], bf16, name="quad",
                                        tag="quad", bufs=2,
                                    )
                                    nc.vector.tensor_tensor(
                                        quad[:, 0:512],
                                        pend_pairs[g - 1][0][:, 0:512],
                                        pair[:, 0:512],
                                        ADD,
                                    )
                                    pend_pairs[g - 1] = None
                                    pend_pairs[g] = None
                                    pend_quads[min(g + 2, ng - 1)] = (
                                        quad, g // 2)
                            off1 = 0
                            for ks in subs:
                                off, w = ow(ks)
                                first, last = ks == 0, ks == nks - 1
                                prhs = pt[:, off1 : off1 + w]
                                off1 += w
                                nc.tensor.matmul(
                                    yp[:, off : off + w],
                                    v_sb[:, ks, :],
                                    prhs,
                                    start=first,
                                    stop=last,
                                )
                                if last_group:
                                    # final (diagonal) group: direct rowsum
                                    # matmuls (executed after pair 0's
                                    # start=True matmul) so nothing is
                                    # deferred across the head boundary.
                                    nc.tensor.matmul(
                                        rp_[:, off : off + w],
                                        ones_sb[:],
                                        prhs,
                                        start=False,
                                        stop=(ks == nks - 1),
                                    )
                        # 1/rowsum (~18 bits; rowsum >= 1 so no edge cases)
                        rinv = nrm_pool.tile([P, 512], f32, name="rinv",
                                             tag="rinv")
                        nc.vector.reciprocal_approx_fast(rinv[:], rp_[:])
                        nc.vector.tensor_tensor(
                            y_sb[h][:, q0 : q0 + 512], yp[:], rinv[:], MULT
                        )
                    # drain any leftover filler units of the previous chunk
                    for u in units:
                        o_unit(*u, o_ps, o_queues)
            # attention PSUM pools closed: 6 banks free. o_proj tail for the
            # last q-chunk runs from a 4-deep PSUM pool (pure matmul stream;
            # evacuation fully hidden), stores on sync/scalar only (gpsimd
            # issues nothing this late - its software-DGE drain is ~7.6us).
            # The first units run from the still-open o_ps pool: the tail
            # pool's allocation boundary waits on the attention pools'
            # release, and these units bridge that wait.
            tail_queues = (nc.sync, nc.scalar)
            tail_units = make_units(NQC - 1)
            # lead-in: the first two units' h0-h2 matmuls only need heads
            # that finished before the last one - they run while the final
            # head's rowsum-reciprocal/normalize chain completes on DVE.
            lead_ops = []
            for aqu, ct, qb in tail_units[0:2]:
                op = o_ps.tile([P, 512], f32, name="op", tag="op")
                for h in range(GQ - 1):
                    nc.tensor.matmul(
                        op[:],
                        y_sb[h][:, qb * P : (qb + 1) * P],
                        wo_sb[:, h, ct, :],
                        start=(h == 0),
                        stop=False,
                    )
                lead_ops.append((op, ct, qb))
            for op, ct, qb in lead_ops:
                nc.tensor.matmul(
                    op[:],
                    y_sb[GQ - 1][:, qb * P : (qb + 1) * P],
                    wo_sb[:, GQ - 1, ct, :],
                    start=False,
                    stop=True,
                )
                o_finish(op, ct, qb, tail_queues)
            with tc.tile_pool(name="o_tail_ps", bufs=4, space="PSUM") as o_tail:
                for u in tail_units[2:]:
                    o_unit(*u, o_tail, tail_queues)

    nc.compile()
    return nc


def _bf16(a):
    import ml_dtypes

    return np.ascontiguousarray(np.asarray(a, dtype=np.float32)).astype(
        ml_dtypes.bfloat16
    )


def make_in_maps(x, wq, wk, wv, wo, T=T_FULL):
    """Per-core input dicts for run_bass_kernel_spmd."""
    cosT, sinT = _rope_tables(T)
    tri = _tri128()
    onesm = np.ones((P, P), dtype=np.float32)
    rotm = _rot_lhsT()

    xts = [_bf16(x[b].T) for b in range(B)]
    cosT, sinT, tri, onesm, rotm = map(_bf16, (cosT, sinT, tri, onesm, rotm))
    in_maps = []
    for core in range(NCORES):
        b, g = core // 4, core % 4
        wkv = np.concatenate(
            (wk[:, D * g : D * (g + 1)], wv[:, D * g : D * (g + 1)]), axis=1
        )
        in_maps.append(
            {
                "xt": xts[b],
                "wq": _bf16(wq[:, 512 * g : 512 * (g + 1)]),
                "wkv": _bf16(wkv),
                "wo": _bf16(wo[512 * g : 512 * (g + 1), :]),
                "cosT": cosT,
                "sinT": sinT,
                "trim": tri,
                "onesm": onesm,
                "rotm": rotm,
            }
        )
    return in_maps


_NC_CACHE = {}


def _get_nc(T=T_FULL):
    if T not in _NC_CACHE:
        _NC_CACHE[T] = build_nc(T)
    return _NC_CACHE[T]


def run(inputs, trace=False):
    """Run on 8 NeuronCores. Returns (full_output, BassKernelResults)."""
    from concourse.bass_utils import run_bass_kernel_spmd

    x = np.asarray(inputs["x"], dtype=np.float32)
    in_maps = make_in_maps(
        x,
        np.asarray(inputs["wq"], dtype=np.float32),
        np.asarray(inputs["wk"], dtype=np.float32),
        np.asarray(inputs["wv"], dtype=np.float32),
        np.asarray(inputs["wo"], dtype=np.float32),
    )
    nc = _get_nc()
    res = run_bass_kernel_spmd(nc, in_maps, list(range(NCORES)), trace=trace)
    outs = res.results
    full = np.zeros((B, T_FULL, C_DIM), dtype=np.float32)
    for core in range(NCORES):
        full[core // 4] += np.asarray(outs[core]["out"], dtype=np.float32)
    return full, res


def kernel(**inputs):
    full, _ = run(inputs, trace=False)
    return full
